# revision 39
# baseline (speedup 1.0000x reference)
"""Trainium2 Bass kernel for nn_AltAttention (dense transformer attention block).

Reference computation (B=4, S=2048, D=512, H=8, Dh=64):
    qkv  = hidden @ W_qkv + b_qkv                      -> q, k, v per head
    attn = softmax(q k^T * D**-0.5 + alibi, masked)
    out  = (attn @ v) @ W_proj + b_proj

Sharding: one head per NeuronCore (8 heads = 8 cores). Each core computes
q/k/v for its head from the full hidden states, runs attention with
transposed score tiles [ks, qs], applies the proj matmul on-chip, and
writes a partial projection output. The host sums the 8 partials (the
tensor-parallel all-reduce); b_proj rides row 0 of core 0's wproj_aug.

Phase-2 structure (per 512-query block):
  - alibi^T is copied (bf16 SBUF -> f32 PSUM) into the score PSUM tile
    before the score matmul ACCUMULATES on top with start=False. For
    `n_pe` of the 8 tile-pairs per block the copy is done by PE identity
    matmuls (proper start=True group openers); for the rest it is a DVE
    tensor_copy and the score matmul relies on pre-primed PSUM
    has_written bits (dummy start=True matmuls at program start) with
    skip_group_check. Accumulating matmuls run ~2x faster than isolated
    start/stop ones on HW, and the alibi add costs no separate
    elementwise pass (also no host-side exp(alibi) precompute).
  - ACT exp reads s+alibi straight from PSUM -> p (bf16 SBUF).
  - attn@V accumulates 16 k-tiles into x_ps; a ones-column in the V
    operand makes row 0 of x_ps the softmax denominators.
  - Normalization is folded into the PSUM->SBUF eviction of the proj
    output: sums row -> 4 PE transposes -> [128,4] PSUM -> DVE reciprocal
    -> per-partition scale on eviction (ACT for `n_act_ev` of the 4
    m-blocks, DVE for the rest). The b_proj bias-row multiplier is the
    bf16 sums row of xs_sb: sums*b_proj/sums = b_proj.
  - scores use a duplicated-K trick: q^T and k^T stored twice along the
    partition dim so the Dh=64 contraction runs as K=128; the doubled sum
    is compensated in the host-folded weight scale.
  - PSUM (8 banks): 3 double-bank score buffers (triple buffering keeps
    the inject->score->exp cross-engine chain throughput-bound instead of
    latency-bound), 1 bank shared attn-accumulator, 1 bank rotating
    proj-output/recip-transpose pool.
"""

import sys

sys.path.insert(0, "/opt/trn_rl_repo")

import numpy as np
import ml_dtypes

import concourse.bass as bass
import concourse.tile as tile
from concourse import bacc, mybir
from concourse.bass_utils import run_bass_kernel_spmd

BF16 = mybir.dt.bfloat16
F32 = mybir.dt.float32
NP_BF16 = ml_dtypes.bfloat16

B, S, D, H = 4, 2048, 512, 8
Dh = D // H  # 64
BS = B * S  # 8192
P = 128
NKT = S // P  # 16 ks tiles per batch
NQB = S // 512  # 4 query blocks of 512 per batch
NSC = S // 512  # 4 s-chunks of 512 per batch (qkv phase)
SCALE = D ** (-0.5)


def build_program(eb: int, repeat: int = 1, phases=(1, 2), skel=False,
                  n_pe: int = 2, n_act_ev: int = 2, n_act_inj: int = 1,
                  spool3: bool = True):
    """Build the per-core Bass program. eb = number of alibi slices
    (1 when the attention mask is all ones, B otherwise)."""
    nc = bacc.Bacc("TRN2", target_bir_lowering=False, debug=False, num_devices=H)

    hiddenT = nc.dram_tensor("hiddenT", [D, BS], BF16, kind="ExternalInput")
    # aT layout: [eb, NQB, 128, NKT, 512] so each (e, qb) slice is one
    # contiguous 2 MB DMA; values are alibi^T (masked keys at -30)
    aT = nc.dram_tensor("aT", [eb, NQB, P, NKT, 512], BF16,
                        kind="ExternalInput")
    wqk = nc.dram_tensor("wqk", [4, P, P], BF16, kind="ExternalInput")
    bqk = nc.dram_tensor("bqk", [P, 1], F32, kind="ExternalInput")
    wv = nc.dram_tensor("wv", [4, P, Dh], BF16, kind="ExternalInput")
    bv = nc.dram_tensor("bv", [Dh, 1], F32, kind="ExternalInput")
    wproj = nc.dram_tensor("wproj", [Dh + 1, D], BF16, kind="ExternalInput")
    ident = nc.dram_tensor("ident", [P, P], BF16, kind="ExternalInput")
    part = nc.dram_tensor("part", [BS, D], F32, kind="ExternalOutput")

    hT_re = hiddenT[:].rearrange("(c p) s -> p c s", p=P)  # [128, 4, 8192]

    with tile.TileContext(nc) as tc:
        with tc.tile_pool(name="consts", bufs=1) as consts, \
             tc.tile_pool(name="persist", bufs=1) as persist:
            wqk_sb = consts.tile([P, 4, P], BF16)
            nc.sync.dma_start(wqk_sb[:], wqk[:].rearrange("c p m -> p c m"))
            wv_sb = consts.tile([P, 4, Dh], BF16)
            nc.sync.dma_start(wv_sb[:], wv[:].rearrange("c p m -> p c m"))
            bqk_sb = consts.tile([P, 1], F32)
            nc.sync.dma_start(bqk_sb[:], bqk[:])
            bv_sb = consts.tile([Dh, 1], F32)
            nc.sync.dma_start(bv_sb[:], bv[:])
            wproj_sb = consts.tile([Dh + 1, D], BF16)
            nc.sync.dma_start(wproj_sb[:], wproj[:])
            ident_sb = consts.tile([P, P], BF16)
            nc.sync.dma_start(ident_sb[:], ident[:])
            ident1 = consts.tile([1, 1], F32)
            nc.vector.memset(ident1[:], 1.0)

            qT2 = persist.tile([P, BS], BF16)  # rows 0:64 qT, 64:128 qT again
            kT2 = persist.tile([P, BS], BF16)
            # padded layout: tile t = [:, t, 63:128]; col 63 = ones (sums row),
            # cols 64:128 = v^T (DMA-transpose needs 128B-aligned dest offsets)
            vaug = persist.tile([P, B * NKT, P], BF16)
            nc.vector.memset(vaug[:, :, Dh - 1 : Dh], 1.0)
            vt_all = persist.tile([Dh, B * NSC, 512], BF16)

            if 1 not in phases:
                nc.vector.memset(qT2[:], 0.01)
                nc.vector.memset(kT2[:], 0.01)
                nc.vector.memset(vaug[:], 0.01)
                nc.vector.memset(vaug[:, :, Dh - 1 : Dh], 1.0)

            for rep in range(repeat):
                # ---------------- phase 1: qkv projections ----------------
                if 1 in phases:
                 with tc.tile_pool(name="hpool", bufs=2) as hpool, \
                      tc.tile_pool(name="qkps", bufs=2, space="PSUM") as qkps, \
                      tc.tile_pool(name="vtps", bufs=2, space="PSUM") as vtps:
                    for b in range(B):
                        ht = hpool.tile([P, 4, S], BF16)
                        nc.sync.dma_start(ht[:],
                                          hT_re[:, :, b * S : (b + 1) * S])
                        for sci in range(NSC):
                            col0 = b * S + sci * 512
                            ssl = slice(sci * 512, (sci + 1) * 512)

                            qk_ps = qkps.tile([P, 512], F32)
                            for c in range(4):
                                nc.tensor.matmul(qk_ps[:], wqk_sb[:, c, :],
                                                 ht[:, c, ssl],
                                                 start=(c == 0), stop=(c == 3))
                            sl = slice(col0, col0 + 512)
                            nc.vector.tensor_scalar_add(
                                qT2[0:Dh, sl], qk_ps[0:Dh, :], bqk_sb[0:Dh, :])
                            nc.vector.tensor_scalar_add(
                                kT2[Dh:P, sl], qk_ps[Dh:P, :], bqk_sb[Dh:P, :])

                            vt_ps = vtps.tile([Dh, 512], F32)
                            for c in range(4):
                                nc.tensor.matmul(vt_ps[:], wv_sb[:, c, :],
                                                 ht[:, c, ssl],
                                                 start=(c == 0), stop=(c == 3))
                            i = b * NSC + sci
                            nc.vector.tensor_scalar_add(
                                vt_all[:, i, :], vt_ps[:], bv_sb[:])
                        bsl = slice(b * S, (b + 1) * S)
                        nc.sync.dma_start(qT2[Dh:P, bsl], qT2[0:Dh, bsl])
                        nc.sync.dma_start(kT2[0:Dh, bsl], kT2[Dh:P, bsl])
                    for i in range(B * NSC):
                        nc.sync.dma_start(vaug[:, i * 4 : i * 4 + 4, Dh:P],
                                          vt_all[:, i, :], transpose=True)

                # ---------------- phase 2: attention + proj ----------------
                if 2 in phases:
                 with tc.tile_pool(name="apool", bufs=2) as apool, \
                      tc.tile_pool(name="ppool", bufs=3) as ppool, \
                      tc.tile_pool(name="xssb", bufs=2) as xssb, \
                      tc.tile_pool(name="smsb", bufs=2) as smsb, \
                      tc.tile_pool(name="rsb", bufs=2) as rsb, \
                      tc.tile_pool(name="outpool", bufs=2) as outpool, \
                      tc.tile_pool(name="spool", bufs=(3 if spool3 else 2),
                                   space="PSUM") as spool, \
                      tc.tile_pool(name="xrps", bufs=1, space="PSUM") as xrps, \
                      tc.tile_pool(name="ops", bufs=(1 if spool3 else 3),
                                   space="PSUM") as ops:
                    # prime the two s_ps pool buffers: start=True matmuls set
                    # the PSUM has_written bits so DVE-injected score matmuls
                    # can run start=False and accumulate onto DVE-written alibi
                    if rep == 0:
                        for _ in range(3 if spool3 else 2):
                            s_ps = spool.tile([P, 1024], F32)
                            for j in range(2):
                                nc.tensor.matmul(
                                    s_ps[:, j * 512 : (j + 1) * 512],
                                    wqk_sb[:, 0, :],
                                    wqk_sb[:].rearrange("p c m -> p (c m)"),
                                    start=True, stop=True)
                    def emit_block(qb, b, aT_t):
                        qsl = slice(b * S + qb * 512, b * S + (qb + 1) * 512)
                        x_t = xrps.tile([P, 512], F32, tag="x")
                        x_ps = x_t[0 : Dh + 1, :]
                        for g in range(NKT // 2):
                            s_ps = spool.tile([P, 1024], F32)
                            if g < n_pe:
                                for j in range(2):
                                    nc.tensor.matmul(
                                        s_ps[:, j * 512 : (j + 1) * 512],
                                        ident_sb[:], aT_t[:, 2 * g + j, :],
                                        start=True, stop=False)
                            elif g < n_pe + n_act_inj:
                                nc.scalar.copy(
                                    s_ps[:],
                                    aT_t[:, 2 * g : 2 * g + 2, :].rearrange(
                                        "p a b -> p (a b)"))
                            else:
                                nc.vector.tensor_copy(
                                    s_ps[:],
                                    aT_t[:, 2 * g : 2 * g + 2, :].rearrange(
                                        "p a b -> p (a b)"))
                            for j in range(2):
                                tk = g * 2 + j
                                ksl = slice(b * S + tk * P,
                                            b * S + (tk + 1) * P)
                                nc.tensor.matmul(
                                    s_ps[:, j * 512 : (j + 1) * 512],
                                    kT2[:, ksl], qT2[:, qsl],
                                    start=False, stop=True,
                                    skip_group_check=(g >= n_pe))
                            p_t = ppool.tile([P, 1024], BF16)
                            if not skel:
                                nc.scalar.activation(
                                    p_t[:], s_ps[:],
                                    mybir.ActivationFunctionType.Exp)
                            else:
                                nc.vector.tensor_copy(p_t[:], s_ps[:])
                            for j in range(2):
                                tk = g * 2 + j
                                t = b * NKT + tk
                                nc.tensor.matmul(
                                    x_ps, vaug[:, t, Dh - 1 : P],
                                    p_t[:, j * 512 : (j + 1) * 512],
                                    start=(tk == 0), stop=(tk == NKT - 1))
                        # tail: evict sums (f32) + sums+xs (bf16; row 0 = sums
                        # rides the proj as the b_proj bias-row multiplier)
                        sums_sb = smsb.tile([1, 512], F32)
                        nc.vector.tensor_copy(sums_sb[:], x_t[0:1, :])
                        xs_sb = xssb.tile([Dh + 1, 512], BF16)
                        nc.vector.tensor_copy(xs_sb[:], x_ps)
                        # 4 PE transposes: sums [1,512] -> [128,4] PSUM
                        r_t = ops.tile([P, 512], F32, tag="ps512")
                        r_ps = r_t[:, 0:4]
                        for m in range(4):
                            nc.tensor.transpose(
                                r_ps[:, m : m + 1],
                                sums_sb[0:1, m * P : (m + 1) * P],
                                ident1[:])
                        rT = rsb.tile([P, 4], F32)
                        nc.vector.reciprocal(rT[:], r_ps)
                        out_sb = outpool.tile([P, 4, 512], F32)
                        for m in range(4):
                            out_ps = ops.tile([P, 512], F32, tag="ps512")
                            nc.tensor.matmul(out_ps[:],
                                             xs_sb[:, m * P : (m + 1) * P],
                                             wproj_sb[:],
                                             start=True, stop=True)
                            if m < n_act_ev:
                                nc.scalar.mul(out_sb[:, m, :], out_ps[:],
                                              rT[:, m : m + 1])
                            else:
                                nc.vector.tensor_scalar_mul(
                                    out_sb[:, m, :], out_ps[:],
                                    rT[:, m : m + 1])
                        row0 = b * S + qb * 512
                        nc.gpsimd.dma_start(
                            part[row0 : row0 + 512, :].rearrange(
                                "(m p) d -> p m d", p=P),
                            out_sb[:])

                    for qb in range(NQB):
                        if eb == 1:
                            aT_t = apool.tile([P, NKT, 512], BF16)
                            nc.sync.dma_start(aT_t[:], aT[0, qb])
                        for b in range(B):
                            if eb != 1:
                                aT_t = apool.tile([P, NKT, 512], BF16)
                                nc.sync.dma_start(aT_t[:], aT[b, qb])
                            emit_block(qb, b, aT_t)

    nc.compile()
    return nc


_CACHE = {}


def _get_program(eb: int):
    key = ("prog", eb)
    if key not in _CACHE:
        _CACHE[key] = build_program(eb)
    return _CACHE[key]


def prepare_inputs(hidden_states, attention_mask, alibi_bias, W_qkv, b_qkv,
                   W_proj, b_proj):
    """Host-side prep: transposes, scale folding, bf16 casts.
    Returns (in_maps, eb)."""
    hidden_states = np.asarray(hidden_states, dtype=np.float32)
    attention_mask = np.asarray(attention_mask)
    alibi_bias = np.asarray(alibi_bias, dtype=np.float32)
    W_qkv = np.asarray(W_qkv, dtype=np.float32)
    b_qkv = np.asarray(b_qkv, dtype=np.float32)
    W_proj = np.asarray(W_proj, dtype=np.float32)
    b_proj = np.asarray(b_proj, dtype=np.float32)

    # per-side scale: total scale SCALE, halved once more to undo the
    # duplicated-K (x2) trick in the score matmul
    s_side = np.float32(np.sqrt(SCALE / 2.0))

    hiddenT = np.ascontiguousarray(
        hidden_states.reshape(BS, D).T).astype(NP_BF16)

    mask_trivial = bool(attention_mask.all())
    eb = 1 if mask_trivial else B

    identity = np.eye(P, dtype=NP_BF16)

    def a_layout(m):
        # [S(k), S(q)] -> [NQB, 128, NKT, 512] contiguous per qb slice
        return np.ascontiguousarray(
            m.reshape(NKT, P, NQB, 512).transpose(2, 1, 0, 3))

    a_all = []
    for h in range(H):
        aTh = alibi_bias[0, h].T.astype(NP_BF16)  # [S(k), S(q)]
        if mask_trivial:
            a_all.append(a_layout(aTh)[None])
        else:
            a_all.append(np.stack(
                [a_layout(np.where(attention_mask[bi][:, None], aTh,
                                   NP_BF16(-30.0)))
                 for bi in range(B)]))
    in_maps = []
    for h in range(H):
        # reference reshapes qkv to (B, S, H, 3*Dh) then splits: head h's
        # q/k/v live in columns [h*3*Dh, h*3*Dh + 3*Dh)
        qs = slice(h * 3 * Dh, h * 3 * Dh + Dh)
        ks = slice(h * 3 * Dh + Dh, h * 3 * Dh + 2 * Dh)
        vs = slice(h * 3 * Dh + 2 * Dh, h * 3 * Dh + 3 * Dh)
        wqk_h = np.concatenate([W_qkv[:, qs], W_qkv[:, ks]], axis=1) * s_side
        bqk_h = np.concatenate([b_qkv[qs], b_qkv[ks]]) * s_side
        wv_h = W_qkv[:, vs]
        bv_h = b_qkv[vs]
        wproj_aug = np.concatenate(
            [(b_proj if h == 0 else np.zeros_like(b_proj))[None, :],
             W_proj[h * Dh : (h + 1) * Dh, :]], axis=0)
        in_maps.append({
            "hiddenT": hiddenT,
            "aT": a_all[h],
            "wqk": np.ascontiguousarray(
                wqk_h.reshape(4, P, P).astype(NP_BF16)),
            "bqk": np.ascontiguousarray(bqk_h[:, None]),
            "wv": np.ascontiguousarray(wv_h.reshape(4, P, Dh).astype(NP_BF16)),
            "bv": np.ascontiguousarray(bv_h[:, None]),
            "wproj": wproj_aug.astype(NP_BF16),
            "ident": identity,
        })
    return in_maps, eb


def kernel(**inputs):
    in_maps, eb = prepare_inputs(**inputs)
    nc = _get_program(eb)
    res = run_bass_kernel_spmd(nc, in_maps, list(range(H)))
    out = res.results[0]["part"].astype(np.float32)
    for h in range(1, H):
        out = out + res.results[h]["part"]
    return out.reshape(B, S, D)


# revision 40
# speedup vs baseline: 1.0181x; 1.0181x over previous
"""Trainium2 Bass kernel for nn_AltAttention (dense transformer attention block).

Reference computation (B=4, S=2048, D=512, H=8, Dh=64):
    qkv  = hidden @ W_qkv + b_qkv                      -> q, k, v per head
    attn = softmax(q k^T * D**-0.5 + alibi, masked)
    out  = (attn @ v) @ W_proj + b_proj

Sharding: one head per NeuronCore (8 heads = 8 cores). Each core computes
q/k/v for its head from the full hidden states, runs attention with
transposed score tiles [ks, qs], applies the proj matmul on-chip, and
writes a partial projection output. The host sums the 8 partials (the
tensor-parallel all-reduce); b_proj rides row 0 of core 0's wproj_aug.

Phase-2 structure (per 512-query block):
  - alibi^T is copied (bf16 SBUF -> f32 PSUM) into the score PSUM tile
    before the score matmul ACCUMULATES on top with start=False. For
    `n_pe` of the 8 tile-pairs per block the copy is done by PE identity
    matmuls (proper start=True group openers); for the rest it is a DVE
    tensor_copy and the score matmul relies on pre-primed PSUM
    has_written bits (dummy start=True matmuls at program start) with
    skip_group_check. Accumulating matmuls run ~2x faster than isolated
    start/stop ones on HW, and the alibi add costs no separate
    elementwise pass (also no host-side exp(alibi) precompute).
  - ACT exp reads s+alibi straight from PSUM -> p (bf16 SBUF).
  - attn@V accumulates 16 k-tiles into x_ps; a ones-column in the V
    operand makes row 0 of x_ps the softmax denominators.
  - Normalization is folded into the PSUM->SBUF eviction of the proj
    output: sums row -> 4 PE transposes -> [128,4] PSUM -> DVE reciprocal
    -> per-partition scale on eviction (ACT for `n_act_ev` of the 4
    m-blocks, DVE for the rest). The b_proj bias-row multiplier is the
    bf16 sums row of xs_sb: sums*b_proj/sums = b_proj.
  - scores use a duplicated-K trick: q^T and k^T stored twice along the
    partition dim so the Dh=64 contraction runs as K=128; the doubled sum
    is compensated in the host-folded weight scale.
  - PSUM (8 banks): 3 double-bank score buffers (triple buffering keeps
    the inject->score->exp cross-engine chain throughput-bound instead of
    latency-bound), 1 bank shared attn-accumulator, 1 bank rotating
    proj-output/recip-transpose pool.
"""

import sys

sys.path.insert(0, "/opt/trn_rl_repo")

import numpy as np
import ml_dtypes

import concourse.bass as bass
import concourse.tile as tile
from concourse import bacc, mybir
from concourse.bass_utils import run_bass_kernel_spmd

BF16 = mybir.dt.bfloat16
F32 = mybir.dt.float32
NP_BF16 = ml_dtypes.bfloat16

B, S, D, H = 4, 2048, 512, 8
Dh = D // H  # 64
BS = B * S  # 8192
P = 128
NKT = S // P  # 16 ks tiles per batch
NQB = S // 512  # 4 query blocks of 512 per batch
NSC = S // 512  # 4 s-chunks of 512 per batch (qkv phase)
SCALE = D ** (-0.5)


def build_program(eb: int, repeat: int = 1, phases=(1, 2), skel=False,
                  n_pe: int = 2, n_act_ev: int = 2, n_act_inj: int = 1,
                  spool3: bool = True):
    """Build the per-core Bass program. eb = number of alibi slices
    (1 when the attention mask is all ones, B otherwise)."""
    nc = bacc.Bacc("TRN2", target_bir_lowering=False, debug=False, num_devices=H)

    hiddenT = nc.dram_tensor("hiddenT", [D, BS], BF16, kind="ExternalInput")
    # aT layout: [eb, NQB, 128, NKT, 512] so each (e, qb) slice is one
    # contiguous 2 MB DMA; values are alibi^T (masked keys at -30)
    aT = nc.dram_tensor("aT", [eb, NQB, P, NKT, 512], BF16,
                        kind="ExternalInput")
    wqk = nc.dram_tensor("wqk", [4, P, P], BF16, kind="ExternalInput")
    bqk = nc.dram_tensor("bqk", [P, 1], F32, kind="ExternalInput")
    wv = nc.dram_tensor("wv", [4, P, Dh], BF16, kind="ExternalInput")
    bv = nc.dram_tensor("bv", [Dh, 1], F32, kind="ExternalInput")
    wproj = nc.dram_tensor("wproj", [Dh + 1, D], BF16, kind="ExternalInput")
    ident = nc.dram_tensor("ident", [P, P], BF16, kind="ExternalInput")
    part = nc.dram_tensor("part", [BS, D], F32, kind="ExternalOutput")

    hT_re = hiddenT[:].rearrange("(c p) s -> p c s", p=P)  # [128, 4, 8192]

    with tile.TileContext(nc) as tc:
        with tc.tile_pool(name="consts", bufs=1) as consts, \
             tc.tile_pool(name="persist", bufs=1) as persist:
            wqk_sb = consts.tile([P, 4, P], BF16)
            nc.sync.dma_start(wqk_sb[:], wqk[:].rearrange("c p m -> p c m"))
            wv_sb = consts.tile([P, 4, Dh], BF16)
            nc.sync.dma_start(wv_sb[:], wv[:].rearrange("c p m -> p c m"))
            bqk_sb = consts.tile([P, 1], F32)
            nc.sync.dma_start(bqk_sb[:], bqk[:])
            bv_sb = consts.tile([Dh, 1], F32)
            nc.sync.dma_start(bv_sb[:], bv[:])
            wproj_sb = consts.tile([Dh + 1, D], BF16)
            nc.sync.dma_start(wproj_sb[:], wproj[:])
            ident_sb = consts.tile([P, P], BF16)
            nc.sync.dma_start(ident_sb[:], ident[:])
            ident1 = consts.tile([1, 1], F32)
            nc.vector.memset(ident1[:], 1.0)

            qT2 = persist.tile([P, BS], BF16)  # rows 0:64 qT, 64:128 qT again
            kT2 = persist.tile([P, BS], BF16)
            # padded layout: tile t = [:, t, 63:128]; col 63 = ones (sums row),
            # cols 64:128 = v^T (DMA-transpose needs 128B-aligned dest offsets)
            vaug = persist.tile([P, B * NKT, P], BF16)
            nc.vector.memset(vaug[:, :, Dh - 1 : Dh], 1.0)
            vt_all = persist.tile([Dh, B * NSC, 512], BF16)

            if 1 not in phases:
                nc.vector.memset(qT2[:], 0.01)
                nc.vector.memset(kT2[:], 0.01)
                nc.vector.memset(vaug[:], 0.01)
                nc.vector.memset(vaug[:, :, Dh - 1 : Dh], 1.0)

            for rep in range(repeat):
                # ---------------- phase 1: qkv projections ----------------
                if 1 in phases:
                 with tc.tile_pool(name="hpool", bufs=2) as hpool, \
                      tc.tile_pool(name="qkps", bufs=2, space="PSUM") as qkps, \
                      tc.tile_pool(name="vtps", bufs=2, space="PSUM") as vtps:
                    for b in range(B):
                        ht = hpool.tile([P, 4, S], BF16)
                        nc.sync.dma_start(ht[:],
                                          hT_re[:, :, b * S : (b + 1) * S])
                        for sci in range(NSC):
                            col0 = b * S + sci * 512
                            ssl = slice(sci * 512, (sci + 1) * 512)

                            qk_ps = qkps.tile([P, 512], F32)
                            for c in range(4):
                                nc.tensor.matmul(qk_ps[:], wqk_sb[:, c, :],
                                                 ht[:, c, ssl],
                                                 start=(c == 0), stop=(c == 3))
                            sl = slice(col0, col0 + 512)
                            nc.vector.tensor_scalar_add(
                                qT2[0:Dh, sl], qk_ps[0:Dh, :], bqk_sb[0:Dh, :])
                            nc.vector.tensor_scalar_add(
                                kT2[Dh:P, sl], qk_ps[Dh:P, :], bqk_sb[Dh:P, :])

                            vt_ps = vtps.tile([Dh, 512], F32)
                            for c in range(4):
                                nc.tensor.matmul(vt_ps[:], wv_sb[:, c, :],
                                                 ht[:, c, ssl],
                                                 start=(c == 0), stop=(c == 3))
                            i = b * NSC + sci
                            nc.vector.tensor_scalar_add(
                                vt_all[:, i, :], vt_ps[:], bv_sb[:])
                        bsl = slice(b * S, (b + 1) * S)
                        nc.gpsimd.dma_start(qT2[Dh:P, bsl], qT2[0:Dh, bsl])
                        nc.gpsimd.dma_start(kT2[0:Dh, bsl], kT2[Dh:P, bsl])
                        # per-batch transposes: batch b's attention unblocks
                        # without waiting for the remaining batches' qkv
                        for i in range(b * NSC, (b + 1) * NSC):
                            nc.sync.dma_start(vaug[:, i * 4 : i * 4 + 4, Dh:P],
                                              vt_all[:, i, :], transpose=True)

                # ---------------- phase 2: attention + proj ----------------
                if 2 in phases:
                 with tc.tile_pool(name="apool", bufs=2) as apool, \
                      tc.tile_pool(name="ppool", bufs=3) as ppool, \
                      tc.tile_pool(name="xssb", bufs=2) as xssb, \
                      tc.tile_pool(name="smsb", bufs=2) as smsb, \
                      tc.tile_pool(name="rsb", bufs=2) as rsb, \
                      tc.tile_pool(name="outpool", bufs=2) as outpool, \
                      tc.tile_pool(name="spool", bufs=(3 if spool3 else 2),
                                   space="PSUM") as spool, \
                      tc.tile_pool(name="xrps", bufs=1, space="PSUM") as xrps, \
                      tc.tile_pool(name="ops", bufs=(1 if spool3 else 3),
                                   space="PSUM") as ops:
                    # prime the two s_ps pool buffers: start=True matmuls set
                    # the PSUM has_written bits so DVE-injected score matmuls
                    # can run start=False and accumulate onto DVE-written alibi
                    if rep == 0:
                        for _ in range(3 if spool3 else 2):
                            s_ps = spool.tile([P, 1024], F32)
                            for j in range(2):
                                nc.tensor.matmul(
                                    s_ps[:, j * 512 : (j + 1) * 512],
                                    wqk_sb[:, 0, :],
                                    wqk_sb[:].rearrange("p c m -> p (c m)"),
                                    start=True, stop=True)
                    def emit_block(qb, b, aT_t):
                        qsl = slice(b * S + qb * 512, b * S + (qb + 1) * 512)
                        x_t = xrps.tile([P, 512], F32, tag="x")
                        x_ps = x_t[0 : Dh + 1, :]
                        for g in range(NKT // 2):
                            s_ps = spool.tile([P, 1024], F32)
                            if g < n_pe:
                                for j in range(2):
                                    nc.tensor.matmul(
                                        s_ps[:, j * 512 : (j + 1) * 512],
                                        ident_sb[:], aT_t[:, 2 * g + j, :],
                                        start=True, stop=False)
                            elif g < n_pe + n_act_inj:
                                nc.scalar.copy(
                                    s_ps[:],
                                    aT_t[:, 2 * g : 2 * g + 2, :].rearrange(
                                        "p a b -> p (a b)"))
                            else:
                                nc.vector.tensor_copy(
                                    s_ps[:],
                                    aT_t[:, 2 * g : 2 * g + 2, :].rearrange(
                                        "p a b -> p (a b)"))
                            for j in range(2):
                                tk = g * 2 + j
                                ksl = slice(b * S + tk * P,
                                            b * S + (tk + 1) * P)
                                nc.tensor.matmul(
                                    s_ps[:, j * 512 : (j + 1) * 512],
                                    kT2[:, ksl], qT2[:, qsl],
                                    start=False, stop=True,
                                    skip_group_check=(g >= n_pe))
                            p_t = ppool.tile([P, 1024], BF16)
                            if not skel:
                                nc.scalar.activation(
                                    p_t[:], s_ps[:],
                                    mybir.ActivationFunctionType.Exp)
                            else:
                                nc.vector.tensor_copy(p_t[:], s_ps[:])
                            for j in range(2):
                                tk = g * 2 + j
                                t = b * NKT + tk
                                nc.tensor.matmul(
                                    x_ps, vaug[:, t, Dh - 1 : P],
                                    p_t[:, j * 512 : (j + 1) * 512],
                                    start=(tk == 0), stop=(tk == NKT - 1))
                        # tail: evict sums (f32) + sums+xs (bf16; row 0 = sums
                        # rides the proj as the b_proj bias-row multiplier)
                        sums_sb = smsb.tile([1, 512], F32)
                        nc.vector.tensor_copy(sums_sb[:], x_t[0:1, :])
                        xs_sb = xssb.tile([Dh + 1, 512], BF16)
                        nc.vector.tensor_copy(xs_sb[:], x_ps)
                        # 4 PE transposes: sums [1,512] -> [128,4] PSUM
                        r_t = ops.tile([P, 512], F32, tag="ps512")
                        r_ps = r_t[:, 0:4]
                        for m in range(4):
                            nc.tensor.transpose(
                                r_ps[:, m : m + 1],
                                sums_sb[0:1, m * P : (m + 1) * P],
                                ident1[:])
                        rT = rsb.tile([P, 4], F32)
                        nc.vector.reciprocal(rT[:], r_ps)
                        out_sb = outpool.tile([P, 4, 512], F32)
                        for m in range(4):
                            out_ps = ops.tile([P, 512], F32, tag="ps512")
                            nc.tensor.matmul(out_ps[:],
                                             xs_sb[:, m * P : (m + 1) * P],
                                             wproj_sb[:],
                                             start=True, stop=True)
                            if m < n_act_ev:
                                nc.scalar.mul(out_sb[:, m, :], out_ps[:],
                                              rT[:, m : m + 1])
                            else:
                                nc.vector.tensor_scalar_mul(
                                    out_sb[:, m, :], out_ps[:],
                                    rT[:, m : m + 1])
                        row0 = b * S + qb * 512
                        nc.gpsimd.dma_start(
                            part[row0 : row0 + 512, :].rearrange(
                                "(m p) d -> p m d", p=P),
                            out_sb[:])

                    for qb in range(NQB):
                        if eb == 1:
                            aT_t = apool.tile([P, NKT, 512], BF16)
                            nc.sync.dma_start(aT_t[:], aT[0, qb])
                        for b in range(B):
                            if eb != 1:
                                aT_t = apool.tile([P, NKT, 512], BF16)
                                nc.sync.dma_start(aT_t[:], aT[b, qb])
                            emit_block(qb, b, aT_t)

    nc.compile()
    return nc


_CACHE = {}


def _get_program(eb: int):
    key = ("prog", eb)
    if key not in _CACHE:
        _CACHE[key] = build_program(eb)
    return _CACHE[key]


def prepare_inputs(hidden_states, attention_mask, alibi_bias, W_qkv, b_qkv,
                   W_proj, b_proj):
    """Host-side prep: transposes, scale folding, bf16 casts.
    Returns (in_maps, eb)."""
    hidden_states = np.asarray(hidden_states, dtype=np.float32)
    attention_mask = np.asarray(attention_mask)
    alibi_bias = np.asarray(alibi_bias, dtype=np.float32)
    W_qkv = np.asarray(W_qkv, dtype=np.float32)
    b_qkv = np.asarray(b_qkv, dtype=np.float32)
    W_proj = np.asarray(W_proj, dtype=np.float32)
    b_proj = np.asarray(b_proj, dtype=np.float32)

    # per-side scale: total scale SCALE, halved once more to undo the
    # duplicated-K (x2) trick in the score matmul
    s_side = np.float32(np.sqrt(SCALE / 2.0))

    hiddenT = np.ascontiguousarray(
        hidden_states.reshape(BS, D).T).astype(NP_BF16)

    mask_trivial = bool(attention_mask.all())
    eb = 1 if mask_trivial else B

    identity = np.eye(P, dtype=NP_BF16)

    def a_layout(m):
        # [S(k), S(q)] -> [NQB, 128, NKT, 512] contiguous per qb slice
        return np.ascontiguousarray(
            m.reshape(NKT, P, NQB, 512).transpose(2, 1, 0, 3))

    a_all = []
    for h in range(H):
        aTh = alibi_bias[0, h].T.astype(NP_BF16)  # [S(k), S(q)]
        if mask_trivial:
            a_all.append(a_layout(aTh)[None])
        else:
            a_all.append(np.stack(
                [a_layout(np.where(attention_mask[bi][:, None], aTh,
                                   NP_BF16(-30.0)))
                 for bi in range(B)]))
    in_maps = []
    for h in range(H):
        # reference reshapes qkv to (B, S, H, 3*Dh) then splits: head h's
        # q/k/v live in columns [h*3*Dh, h*3*Dh + 3*Dh)
        qs = slice(h * 3 * Dh, h * 3 * Dh + Dh)
        ks = slice(h * 3 * Dh + Dh, h * 3 * Dh + 2 * Dh)
        vs = slice(h * 3 * Dh + 2 * Dh, h * 3 * Dh + 3 * Dh)
        wqk_h = np.concatenate([W_qkv[:, qs], W_qkv[:, ks]], axis=1) * s_side
        bqk_h = np.concatenate([b_qkv[qs], b_qkv[ks]]) * s_side
        wv_h = W_qkv[:, vs]
        bv_h = b_qkv[vs]
        wproj_aug = np.concatenate(
            [(b_proj if h == 0 else np.zeros_like(b_proj))[None, :],
             W_proj[h * Dh : (h + 1) * Dh, :]], axis=0)
        in_maps.append({
            "hiddenT": hiddenT,
            "aT": a_all[h],
            "wqk": np.ascontiguousarray(
                wqk_h.reshape(4, P, P).astype(NP_BF16)),
            "bqk": np.ascontiguousarray(bqk_h[:, None]),
            "wv": np.ascontiguousarray(wv_h.reshape(4, P, Dh).astype(NP_BF16)),
            "bv": np.ascontiguousarray(bv_h[:, None]),
            "wproj": wproj_aug.astype(NP_BF16),
            "ident": identity,
        })
    return in_maps, eb


def kernel(**inputs):
    in_maps, eb = prepare_inputs(**inputs)
    nc = _get_program(eb)
    res = run_bass_kernel_spmd(nc, in_maps, list(range(H)))
    out = res.results[0]["part"].astype(np.float32)
    for h in range(1, H):
        out = out + res.results[h]["part"]
    return out.reshape(B, S, D)


# revision 48
# speedup vs baseline: 1.1509x; 1.1304x over previous
"""Trainium2 Bass kernel for nn_AltAttention (dense transformer attention block).

Reference computation (B=4, S=2048, D=512, H=8, Dh=64):
    qkv  = hidden @ W_qkv + b_qkv                      -> q, k, v per head
    attn = softmax(q k^T * D**-0.5 + alibi, masked)
    out  = (attn @ v) @ W_proj + b_proj

Sharding: one head per NeuronCore (8 heads = 8 cores). Each core computes
q/k/v for its head from the full hidden states, runs attention with
transposed score tiles [ks, qs], applies the proj matmul on-chip, and
writes a partial projection output. The host sums the 8 partials (the
tensor-parallel all-reduce); b_proj rides row 0 of core 0's wproj_aug.

Phase-2 structure (per 512-query block):
  - alibi^T is copied (bf16 SBUF -> f32 PSUM) into the score PSUM tile
    before the score matmul ACCUMULATES on top with start=False. For
    `n_pe` of the 8 tile-pairs per block the copy is done by PE identity
    matmuls (proper start=True group openers); for the rest it is a DVE
    tensor_copy and the score matmul relies on pre-primed PSUM
    has_written bits (dummy start=True matmuls at program start) with
    skip_group_check. Accumulating matmuls run ~2x faster than isolated
    start/stop ones on HW, and the alibi add costs no separate
    elementwise pass (also no host-side exp(alibi) precompute).
  - ACT exp reads s+alibi straight from PSUM -> p (bf16 SBUF).
  - attn@V accumulates 16 k-tiles into x_ps; a ones-column in the V
    operand makes row 0 of x_ps the softmax denominators.
  - Normalization is folded into the PSUM->SBUF eviction of the proj
    output: sums row -> 4 PE transposes -> [128,4] PSUM -> DVE reciprocal
    -> per-partition scale on eviction (ACT for `n_act_ev` of the 4
    m-blocks, DVE for the rest). The b_proj bias-row multiplier is the
    bf16 sums row of xs_sb: sums*b_proj/sums = b_proj.
  - scores use a duplicated-K trick: q^T and k^T stored twice along the
    partition dim so the Dh=64 contraction runs as K=128; the doubled sum
    is compensated in the host-folded weight scale.
  - PSUM (8 banks): 3 double-bank score buffers (triple buffering keeps
    the inject->score->exp cross-engine chain throughput-bound instead of
    latency-bound), 1 bank shared attn-accumulator, 1 bank rotating
    proj-output/recip-transpose pool.
"""

import sys

sys.path.insert(0, "/opt/trn_rl_repo")

import numpy as np
import ml_dtypes

import concourse.bass as bass
import concourse.tile as tile
from concourse import bacc, mybir
from concourse.bass_utils import run_bass_kernel_spmd

BF16 = mybir.dt.bfloat16
F32 = mybir.dt.float32
NP_BF16 = ml_dtypes.bfloat16

B, S, D, H = 4, 2048, 512, 8
Dh = D // H  # 64
BS = B * S  # 8192
P = 128
NKT = S // P  # 16 ks tiles per batch
NQB = S // 512  # 4 query blocks of 512 per batch
NSC = S // 512  # 4 s-chunks of 512 per batch (qkv phase)
SCALE = D ** (-0.5)


def build_program(eb: int, repeat: int = 1, phases=(1, 2), skel=False,
                  n_pe: int = 2, n_act_ev: int = 2, n_act_inj: int = 1,
                  spool3: bool = True):
    """Build the per-core Bass program. eb = number of alibi slices
    (1 when the attention mask is all ones, B otherwise)."""
    nc = bacc.Bacc("TRN2", target_bir_lowering=False, debug=False, num_devices=H)

    hiddenT = nc.dram_tensor("hiddenT", [D, BS], BF16, kind="ExternalInput")
    # aT layout: [eb, NQB, 128, NKT, 512] so each (e, qb) slice is one
    # contiguous 2 MB DMA; values are alibi^T (masked keys at -30)
    aT = nc.dram_tensor("aT", [eb, NQB, P, NKT, 512], BF16,
                        kind="ExternalInput")
    wqk = nc.dram_tensor("wqk", [4, P, P], BF16, kind="ExternalInput")
    bqk = nc.dram_tensor("bqk", [P, 1], F32, kind="ExternalInput")
    wv = nc.dram_tensor("wv", [4, P, Dh], BF16, kind="ExternalInput")
    bv = nc.dram_tensor("bv", [Dh, 1], F32, kind="ExternalInput")
    wproj = nc.dram_tensor("wproj", [Dh + 1, D], BF16, kind="ExternalInput")
    ident = nc.dram_tensor("ident", [P, P], BF16, kind="ExternalInput")
    part = nc.dram_tensor("part", [BS, D], F32, kind="ExternalOutput")

    hT_re = hiddenT[:].rearrange("(c p) s -> p c s", p=P)  # [128, 4, 8192]

    with tile.TileContext(nc) as tc:
        with tc.tile_pool(name="consts", bufs=1) as consts, \
             tc.tile_pool(name="persist", bufs=1) as persist:
            wqk_sb = consts.tile([P, 4, P], BF16)
            nc.sync.dma_start(wqk_sb[:], wqk[:].rearrange("c p m -> p c m"))
            wv_sb = consts.tile([P, 4, Dh], BF16)
            nc.sync.dma_start(wv_sb[:], wv[:].rearrange("c p m -> p c m"))
            bqk_sb = consts.tile([P, 1], F32)
            nc.sync.dma_start(bqk_sb[:], bqk[:])
            bv_sb = consts.tile([Dh, 1], F32)
            nc.sync.dma_start(bv_sb[:], bv[:])
            wproj_sb = consts.tile([Dh + 1, D], BF16)
            nc.sync.dma_start(wproj_sb[:], wproj[:])
            ident_sb = consts.tile([P, P], BF16)
            nc.sync.dma_start(ident_sb[:], ident[:])
            ident1 = consts.tile([1, 1], F32)
            nc.vector.memset(ident1[:], 1.0)

            qT2 = persist.tile([P, BS], BF16)  # rows 0:64 qT, 64:128 qT again
            kT2 = persist.tile([P, BS], BF16)
            # padded layout: tile t = [:, t, 63:128]; col 63 = ones (sums row),
            # cols 64:128 = v^T (DMA-transpose needs 128B-aligned dest offsets)
            vaug = persist.tile([P, B * NKT, P], BF16)
            nc.vector.memset(vaug[:, :, Dh - 1 : Dh], 1.0)
            vt_all = persist.tile([Dh, B * NSC, 512], BF16)

            if 1 not in phases:
                nc.vector.memset(qT2[:], 0.01)
                nc.vector.memset(kT2[:], 0.01)
                nc.vector.memset(vaug[:], 0.01)
                nc.vector.memset(vaug[:, :, Dh - 1 : Dh], 1.0)

            for rep in range(repeat):
                # ---------------- phase 1: qkv projections ----------------
                if 1 in phases:
                 with tc.tile_pool(name="hpool", bufs=2) as hpool, \
                      tc.tile_pool(name="qkps", bufs=2, space="PSUM") as qkps, \
                      tc.tile_pool(name="vtps", bufs=2, space="PSUM") as vtps:
                    for b in range(B):
                        ht = hpool.tile([P, 4, S], BF16)
                        nc.sync.dma_start(ht[:],
                                          hT_re[:, :, b * S : (b + 1) * S])
                        for sci in range(NSC):
                            col0 = b * S + sci * 512
                            ssl = slice(sci * 512, (sci + 1) * 512)

                            qk_ps = qkps.tile([P, 512], F32)
                            for c in range(4):
                                nc.tensor.matmul(qk_ps[:], wqk_sb[:, c, :],
                                                 ht[:, c, ssl],
                                                 start=(c == 0), stop=(c == 3))
                            sl = slice(col0, col0 + 512)
                            nc.vector.tensor_scalar_add(
                                qT2[0:Dh, sl], qk_ps[0:Dh, :], bqk_sb[0:Dh, :])
                            nc.vector.tensor_scalar_add(
                                kT2[Dh:P, sl], qk_ps[Dh:P, :], bqk_sb[Dh:P, :])

                            vt_ps = vtps.tile([Dh, 512], F32)
                            for c in range(4):
                                nc.tensor.matmul(vt_ps[:], wv_sb[:, c, :],
                                                 ht[:, c, ssl],
                                                 start=(c == 0), stop=(c == 3))
                            i = b * NSC + sci
                            nc.vector.tensor_scalar_add(
                                vt_all[:, i, :], vt_ps[:], bv_sb[:])
                        bsl = slice(b * S, (b + 1) * S)
                        nc.gpsimd.dma_start(qT2[Dh:P, bsl], qT2[0:Dh, bsl])
                        nc.gpsimd.dma_start(kT2[0:Dh, bsl], kT2[Dh:P, bsl])
                        # per-batch transposes: batch b's attention unblocks
                        # without waiting for the remaining batches' qkv
                        for i in range(b * NSC, (b + 1) * NSC):
                            nc.sync.dma_start(vaug[:, i * 4 : i * 4 + 4, Dh:P],
                                              vt_all[:, i, :], transpose=True)

                # ---------------- phase 2: attention + proj ----------------
                if 2 in phases:
                 with tc.tile_pool(name="apool", bufs=2) as apool, \
                      tc.tile_pool(name="ppool", bufs=3) as ppool, \
                      tc.tile_pool(name="xssb", bufs=2) as xssb, \
                      tc.tile_pool(name="smsb", bufs=2) as smsb, \
                      tc.tile_pool(name="rsb", bufs=2) as rsb, \
                      tc.tile_pool(name="outpool", bufs=2) as outpool, \
                      tc.tile_pool(name="spool", bufs=(3 if spool3 else 2),
                                   space="PSUM") as spool, \
                      tc.tile_pool(name="xrps", bufs=1, space="PSUM") as xrps, \
                      tc.tile_pool(name="ops", bufs=(1 if spool3 else 3),
                                   space="PSUM") as ops:
                    # prime the two s_ps pool buffers: start=True matmuls set
                    # the PSUM has_written bits so DVE-injected score matmuls
                    # can run start=False and accumulate onto DVE-written alibi
                    if rep == 0:
                        for _ in range(3 if spool3 else 2):
                            s_ps = spool.tile([P, 1024], F32)
                            for j in range(2):
                                nc.tensor.matmul(
                                    s_ps[:, j * 512 : (j + 1) * 512],
                                    wqk_sb[:, 0, :],
                                    wqk_sb[:].rearrange("p c m -> p (c m)"),
                                    start=True, stop=True)
                    def emit_block(qb, b, aT_t):
                        qsl = slice(b * S + qb * 512, b * S + (qb + 1) * 512)
                        x_t = xrps.tile([P, 512], F32, tag="x")
                        x_ps = x_t[0 : Dh + 1, :]
                        for g in range(NKT // 2):
                            s_ps = spool.tile([P, 1024], F32)
                            if g < n_pe:
                                for j in range(2):
                                    nc.tensor.matmul(
                                        s_ps[:, j * 512 : (j + 1) * 512],
                                        ident_sb[:], aT_t[:, 2 * g + j, :],
                                        start=True, stop=False)
                            elif g < n_pe + n_act_inj:
                                nc.scalar.copy(
                                    s_ps[:],
                                    aT_t[:, 2 * g : 2 * g + 2, :].rearrange(
                                        "p a b -> p (a b)"))
                            else:
                                nc.vector.tensor_copy(
                                    s_ps[:],
                                    aT_t[:, 2 * g : 2 * g + 2, :].rearrange(
                                        "p a b -> p (a b)"))
                            for j in range(2):
                                tk = g * 2 + j
                                ksl = slice(b * S + tk * P,
                                            b * S + (tk + 1) * P)
                                nc.tensor.matmul(
                                    s_ps[:, j * 512 : (j + 1) * 512],
                                    kT2[:, ksl], qT2[:, qsl],
                                    start=False, stop=True,
                                    skip_group_check=(g >= n_pe))
                            p_t = ppool.tile([P, 1024], BF16)
                            if not skel:
                                nc.scalar.activation(
                                    p_t[:], s_ps[:],
                                    mybir.ActivationFunctionType.Exp)
                            else:
                                nc.vector.tensor_copy(p_t[:], s_ps[:])
                            for j in range(2):
                                tk = g * 2 + j
                                t = b * NKT + tk
                                nc.tensor.matmul(
                                    x_ps, vaug[:, t, Dh - 1 : P],
                                    p_t[:, j * 512 : (j + 1) * 512],
                                    start=(tk == 0), stop=(tk == NKT - 1))
                        # tail: evict sums (f32) + sums+xs (bf16; row 0 = sums
                        # rides the proj as the b_proj bias-row multiplier)
                        sums_sb = smsb.tile([1, 512], F32)
                        nc.vector.tensor_copy(sums_sb[:], x_t[0:1, :])
                        xs_sb = xssb.tile([Dh + 1, 512], BF16)
                        nc.vector.tensor_copy(xs_sb[:], x_ps)
                        # 4 PE transposes: sums [1,512] -> [128,4] PSUM
                        r_t = ops.tile([P, 512], F32, tag="ps512")
                        r_ps = r_t[:, 0:4]
                        for m in range(4):
                            nc.tensor.transpose(
                                r_ps[:, m : m + 1],
                                sums_sb[0:1, m * P : (m + 1) * P],
                                ident1[:])
                        rT = rsb.tile([P, 4], F32)
                        nc.vector.reciprocal(rT[:], r_ps)
                        out_sb = outpool.tile([P, 4, 512], F32)
                        for m in range(4):
                            out_ps = ops.tile([P, 512], F32, tag="ps512")
                            nc.tensor.matmul(out_ps[:],
                                             xs_sb[:, m * P : (m + 1) * P],
                                             wproj_sb[:],
                                             start=True, stop=True)
                            if m < n_act_ev:
                                nc.scalar.mul(out_sb[:, m, :], out_ps[:],
                                              rT[:, m : m + 1])
                            else:
                                nc.vector.tensor_scalar_mul(
                                    out_sb[:, m, :], out_ps[:],
                                    rT[:, m : m + 1])
                        row0 = b * S + qb * 512
                        nc.gpsimd.dma_start(
                            part[row0 : row0 + 512, :].rearrange(
                                "(m p) d -> p m d", p=P),
                            out_sb[:])

                    for qb in range(NQB):
                        if eb == 1:
                            aT_t = apool.tile([P, NKT, 512], BF16)
                            nc.sync.dma_start(aT_t[:], aT[0, qb])
                        for b in range(B):
                            if eb != 1:
                                aT_t = apool.tile([P, NKT, 512], BF16)
                                nc.sync.dma_start(aT_t[:], aT[b, qb])
                            emit_block(qb, b, aT_t)

    nc.compile()
    return nc


_CACHE = {}


def _get_program(eb: int):
    key = ("prog", eb)
    if key not in _CACHE:
        _CACHE[key] = build_program(eb)
    return _CACHE[key]


def prepare_inputs(hidden_states, attention_mask, alibi_bias, W_qkv, b_qkv,
                   W_proj, b_proj):
    """Host-side prep: transposes, scale folding, bf16 casts.
    Returns (in_maps, eb)."""
    hidden_states = np.asarray(hidden_states, dtype=np.float32)
    attention_mask = np.asarray(attention_mask)
    alibi_bias = np.asarray(alibi_bias, dtype=np.float32)
    W_qkv = np.asarray(W_qkv, dtype=np.float32)
    b_qkv = np.asarray(b_qkv, dtype=np.float32)
    W_proj = np.asarray(W_proj, dtype=np.float32)
    b_proj = np.asarray(b_proj, dtype=np.float32)

    # per-side scale: total scale SCALE, halved once more to undo the
    # duplicated-K (x2) trick in the score matmul
    s_side = np.float32(np.sqrt(SCALE / 2.0))

    hiddenT = np.ascontiguousarray(
        hidden_states.reshape(BS, D).T).astype(NP_BF16)

    mask_trivial = bool(attention_mask.all())
    eb = 1 if mask_trivial else B

    identity = np.eye(P, dtype=NP_BF16)

    def a_layout(m):
        # [S(k), S(q)] -> [NQB, 128, NKT, 512] contiguous per qb slice
        return np.ascontiguousarray(
            m.reshape(NKT, P, NQB, 512).transpose(2, 1, 0, 3))

    a_all = []
    for h in range(H):
        aTh = alibi_bias[0, h].T.astype(NP_BF16)  # [S(k), S(q)]
        if mask_trivial:
            a_all.append(a_layout(aTh)[None])
        else:
            a_all.append(np.stack(
                [a_layout(np.where(attention_mask[bi][:, None], aTh,
                                   NP_BF16(-30.0)))
                 for bi in range(B)]))
    in_maps = []
    for h in range(H):
        # reference reshapes qkv to (B, S, H, 3*Dh) then splits: head h's
        # q/k/v live in columns [h*3*Dh, h*3*Dh + 3*Dh)
        qs = slice(h * 3 * Dh, h * 3 * Dh + Dh)
        ks = slice(h * 3 * Dh + Dh, h * 3 * Dh + 2 * Dh)
        vs = slice(h * 3 * Dh + 2 * Dh, h * 3 * Dh + 3 * Dh)
        wqk_h = np.concatenate([W_qkv[:, qs], W_qkv[:, ks]], axis=1) * s_side
        bqk_h = np.concatenate([b_qkv[qs], b_qkv[ks]]) * s_side
        wv_h = W_qkv[:, vs]
        bv_h = b_qkv[vs]
        wproj_aug = np.concatenate(
            [(b_proj if h == 0 else np.zeros_like(b_proj))[None, :],
             W_proj[h * Dh : (h + 1) * Dh, :]], axis=0)
        in_maps.append({
            "hiddenT": hiddenT,
            "aT": a_all[h],
            "wqk": np.ascontiguousarray(
                wqk_h.reshape(4, P, P).astype(NP_BF16)),
            "bqk": np.ascontiguousarray(bqk_h[:, None]),
            "wv": np.ascontiguousarray(wv_h.reshape(4, P, Dh).astype(NP_BF16)),
            "bv": np.ascontiguousarray(bv_h[:, None]),
            "wproj": wproj_aug.astype(NP_BF16),
            "ident": identity,
        })
    return in_maps, eb


def kernel(**inputs):
    in_maps, eb = prepare_inputs(**inputs)
    nc = _get_program(eb)
    res = run_bass_kernel_spmd(nc, in_maps, list(range(H)))
    out = res.results[0]["part"].astype(np.float32)
    for h in range(1, H):
        out = out + res.results[h]["part"]
    return out.reshape(B, S, D)


# revision 49
# speedup vs baseline: 2.2605x; 1.9641x over previous
"""Trainium2 Bass kernel for nn_AltAttention (dense transformer attention block).

Reference computation (B=4, S=2048, D=512, H=8, Dh=64):
    qkv  = hidden @ W_qkv + b_qkv                      -> q, k, v per head
    attn = softmax(q k^T * D**-0.5 + alibi, masked)
    out  = (attn @ v) @ W_proj + b_proj

Sharding: one head per NeuronCore (8 heads = 8 cores). Each core computes
q/k/v for its head from the full hidden states, runs attention with
transposed score tiles [ks, qs], applies the proj matmul on-chip, and
writes a partial projection output. The host sums the 8 partials (the
tensor-parallel all-reduce); b_proj rides row 0 of core 0's wproj_aug.

Phase-2 structure (per 512-query block):
  - alibi^T is copied (bf16 SBUF -> f32 PSUM) into the score PSUM tile
    before the score matmul ACCUMULATES on top with start=False. For
    `n_pe` of the 8 tile-pairs per block the copy is done by PE identity
    matmuls (proper start=True group openers); for the rest it is a DVE
    tensor_copy and the score matmul relies on pre-primed PSUM
    has_written bits (dummy start=True matmuls at program start) with
    skip_group_check. Accumulating matmuls run ~2x faster than isolated
    start/stop ones on HW, and the alibi add costs no separate
    elementwise pass (also no host-side exp(alibi) precompute).
  - ACT exp reads s+alibi straight from PSUM -> p (bf16 SBUF).
  - attn@V accumulates 16 k-tiles into x_ps; a ones-column in the V
    operand makes row 0 of x_ps the softmax denominators.
  - Normalization is folded into the PSUM->SBUF eviction of the proj
    output: sums row -> 4 PE transposes -> [128,4] PSUM -> DVE reciprocal
    -> per-partition scale on eviction (ACT for `n_act_ev` of the 4
    m-blocks, DVE for the rest). The b_proj bias-row multiplier is the
    bf16 sums row of xs_sb: sums*b_proj/sums = b_proj.
  - scores use a duplicated-K trick: q^T and k^T stored twice along the
    partition dim so the Dh=64 contraction runs as K=128; the doubled sum
    is compensated in the host-folded weight scale.
  - PSUM (8 banks): 3 double-bank score buffers (triple buffering keeps
    the inject->score->exp cross-engine chain throughput-bound instead of
    latency-bound), 1 bank shared attn-accumulator, 1 bank rotating
    proj-output/recip-transpose pool.
"""

import sys

sys.path.insert(0, "/opt/trn_rl_repo")

import numpy as np
import ml_dtypes

import concourse.bass as bass
import concourse.tile as tile
from concourse import bacc, mybir
from concourse.bass_utils import run_bass_kernel_spmd

BF16 = mybir.dt.bfloat16
F32 = mybir.dt.float32
NP_BF16 = ml_dtypes.bfloat16

B, S, D, H = 4, 2048, 512, 8
Dh = D // H  # 64
BS = B * S  # 8192
P = 128
NKT = S // P  # 16 ks tiles per batch
NQB = S // 512  # 4 query blocks of 512 per batch
NSC = S // 512  # 4 s-chunks of 512 per batch (qkv phase)
SCALE = D ** (-0.5)


def build_program(eb: int, repeat: int = 1, phases=(1, 2), skel=False,
                  n_pe: int = 2, n_act_ev: int = 2, n_act_inj: int = 1,
                  spool3: bool = True):
    """Build the per-core Bass program. eb = number of alibi slices
    (1 when the attention mask is all ones, B otherwise)."""
    nc = bacc.Bacc("TRN2", target_bir_lowering=False, debug=False, num_devices=H)

    hiddenT = nc.dram_tensor("hiddenT", [D, BS], BF16, kind="ExternalInput")
    # aT layout: [eb, NQB, 128, NKT, 512] so each (e, qb) slice is one
    # contiguous 2 MB DMA; values are alibi^T (masked keys at -30)
    aT = nc.dram_tensor("aT", [eb, NQB, P, NKT, 512], BF16,
                        kind="ExternalInput")
    wqk = nc.dram_tensor("wqk", [4, P, P], BF16, kind="ExternalInput")
    bqk = nc.dram_tensor("bqk", [P, 1], F32, kind="ExternalInput")
    wv = nc.dram_tensor("wv", [4, P, Dh], BF16, kind="ExternalInput")
    bv = nc.dram_tensor("bv", [Dh, 1], F32, kind="ExternalInput")
    wproj = nc.dram_tensor("wproj", [Dh + 1, D], BF16, kind="ExternalInput")
    ident = nc.dram_tensor("ident", [P, P], BF16, kind="ExternalInput")
    part = nc.dram_tensor("part", [BS, D], F32, kind="ExternalOutput")

    hT_re = hiddenT[:].rearrange("(c p) s -> p c s", p=P)  # [128, 4, 8192]

    with tile.TileContext(nc) as tc:
        with tc.tile_pool(name="consts", bufs=1) as consts, \
             tc.tile_pool(name="persist", bufs=1) as persist:
            wqk_sb = consts.tile([P, 4, P], BF16)
            nc.sync.dma_start(wqk_sb[:], wqk[:].rearrange("c p m -> p c m"))
            wv_sb = consts.tile([P, 4, Dh], BF16)
            nc.sync.dma_start(wv_sb[:], wv[:].rearrange("c p m -> p c m"))
            bqk_sb = consts.tile([P, 1], F32)
            nc.sync.dma_start(bqk_sb[:], bqk[:])
            bv_sb = consts.tile([Dh, 1], F32)
            nc.sync.dma_start(bv_sb[:], bv[:])
            wproj_sb = consts.tile([Dh + 1, D], BF16)
            nc.sync.dma_start(wproj_sb[:], wproj[:])
            ident_sb = consts.tile([P, P], BF16)
            nc.sync.dma_start(ident_sb[:], ident[:])
            ident1 = consts.tile([1, 1], F32)
            nc.vector.memset(ident1[:], 1.0)

            qT2 = persist.tile([P, BS], BF16)  # rows 0:64 qT, 64:128 qT again
            kT2 = persist.tile([P, BS], BF16)
            # padded layout: tile t = [:, t, 63:128]; col 63 = ones (sums row),
            # cols 64:128 = v^T (DMA-transpose needs 128B-aligned dest offsets)
            vaug = persist.tile([P, B * NKT, P], BF16)
            nc.vector.memset(vaug[:, :, Dh - 1 : Dh], 1.0)
            vt_all = persist.tile([Dh, B * NSC, 512], BF16)

            if 1 not in phases:
                nc.vector.memset(qT2[:], 0.01)
                nc.vector.memset(kT2[:], 0.01)
                nc.vector.memset(vaug[:], 0.01)
                nc.vector.memset(vaug[:, :, Dh - 1 : Dh], 1.0)

            for rep in range(repeat):
                # ---------------- phase 1: qkv projections ----------------
                if 1 in phases:
                 with tc.tile_pool(name="hpool", bufs=2) as hpool, \
                      tc.tile_pool(name="qkps", bufs=2, space="PSUM") as qkps, \
                      tc.tile_pool(name="vtps", bufs=2, space="PSUM") as vtps:
                    for b in range(B):
                        ht = hpool.tile([P, 4, S], BF16)
                        nc.sync.dma_start(ht[:],
                                          hT_re[:, :, b * S : (b + 1) * S])
                        for sci in range(NSC):
                            col0 = b * S + sci * 512
                            ssl = slice(sci * 512, (sci + 1) * 512)

                            qk_ps = qkps.tile([P, 512], F32)
                            for c in range(4):
                                nc.tensor.matmul(qk_ps[:], wqk_sb[:, c, :],
                                                 ht[:, c, ssl],
                                                 start=(c == 0), stop=(c == 3))
                            sl = slice(col0, col0 + 512)
                            nc.vector.tensor_scalar_add(
                                qT2[0:Dh, sl], qk_ps[0:Dh, :], bqk_sb[0:Dh, :])
                            nc.vector.tensor_scalar_add(
                                kT2[Dh:P, sl], qk_ps[Dh:P, :], bqk_sb[Dh:P, :])

                            vt_ps = vtps.tile([Dh, 512], F32)
                            for c in range(4):
                                nc.tensor.matmul(vt_ps[:], wv_sb[:, c, :],
                                                 ht[:, c, ssl],
                                                 start=(c == 0), stop=(c == 3))
                            i = b * NSC + sci
                            nc.vector.tensor_scalar_add(
                                vt_all[:, i, :], vt_ps[:], bv_sb[:])
                        bsl = slice(b * S, (b + 1) * S)
                        nc.gpsimd.dma_start(qT2[Dh:P, bsl], qT2[0:Dh, bsl])
                        nc.gpsimd.dma_start(kT2[0:Dh, bsl], kT2[Dh:P, bsl])
                        # per-batch transposes: batch b's attention unblocks
                        # without waiting for the remaining batches' qkv
                        for i in range(b * NSC, (b + 1) * NSC):
                            nc.sync.dma_start(vaug[:, i * 4 : i * 4 + 4, Dh:P],
                                              vt_all[:, i, :], transpose=True)

                # ---------------- phase 2: attention + proj ----------------
                if 2 in phases:
                 with tc.tile_pool(name="apool", bufs=3) as apool, \
                      tc.tile_pool(name="ppool", bufs=4) as ppool, \
                      tc.tile_pool(name="xssb", bufs=2) as xssb, \
                      tc.tile_pool(name="smsb", bufs=2) as smsb, \
                      tc.tile_pool(name="rsb", bufs=2) as rsb, \
                      tc.tile_pool(name="outpool", bufs=3) as outpool, \
                      tc.tile_pool(name="spool", bufs=(3 if spool3 else 2),
                                   space="PSUM") as spool, \
                      tc.tile_pool(name="xrps", bufs=1, space="PSUM") as xrps, \
                      tc.tile_pool(name="ops", bufs=(1 if spool3 else 3),
                                   space="PSUM") as ops:
                    # prime the two s_ps pool buffers: start=True matmuls set
                    # the PSUM has_written bits so DVE-injected score matmuls
                    # can run start=False and accumulate onto DVE-written alibi
                    if rep == 0:
                        for _ in range(3 if spool3 else 2):
                            s_ps = spool.tile([P, 1024], F32)
                            for j in range(2):
                                nc.tensor.matmul(
                                    s_ps[:, j * 512 : (j + 1) * 512],
                                    wqk_sb[:, 0, :],
                                    wqk_sb[:].rearrange("p c m -> p (c m)"),
                                    start=True, stop=True)
                    def emit_block(qb, b, aT_t):
                        qsl = slice(b * S + qb * 512, b * S + (qb + 1) * 512)
                        x_t = xrps.tile([P, 512], F32, tag="x")
                        x_ps = x_t[0 : Dh + 1, :]
                        for g in range(NKT // 2):
                            s_ps = spool.tile([P, 1024], F32)
                            if g < n_pe:
                                for j in range(2):
                                    nc.tensor.matmul(
                                        s_ps[:, j * 512 : (j + 1) * 512],
                                        ident_sb[:], aT_t[:, 2 * g + j, :],
                                        start=True, stop=False)
                            elif g < n_pe + n_act_inj:
                                nc.scalar.copy(
                                    s_ps[:],
                                    aT_t[:, 2 * g : 2 * g + 2, :].rearrange(
                                        "p a b -> p (a b)"))
                            else:
                                nc.vector.tensor_copy(
                                    s_ps[:],
                                    aT_t[:, 2 * g : 2 * g + 2, :].rearrange(
                                        "p a b -> p (a b)"))
                            for j in range(2):
                                tk = g * 2 + j
                                ksl = slice(b * S + tk * P,
                                            b * S + (tk + 1) * P)
                                nc.tensor.matmul(
                                    s_ps[:, j * 512 : (j + 1) * 512],
                                    kT2[:, ksl], qT2[:, qsl],
                                    start=False, stop=True,
                                    skip_group_check=(g >= n_pe))
                            p_t = ppool.tile([P, 1024], BF16)
                            if not skel:
                                nc.scalar.activation(
                                    p_t[:], s_ps[:],
                                    mybir.ActivationFunctionType.Exp)
                            else:
                                nc.vector.tensor_copy(p_t[:], s_ps[:])
                            for j in range(2):
                                tk = g * 2 + j
                                t = b * NKT + tk
                                nc.tensor.matmul(
                                    x_ps, vaug[:, t, Dh - 1 : P],
                                    p_t[:, j * 512 : (j + 1) * 512],
                                    start=(tk == 0), stop=(tk == NKT - 1))
                        # tail: evict sums (f32) + sums+xs (bf16; row 0 = sums
                        # rides the proj as the b_proj bias-row multiplier)
                        sums_sb = smsb.tile([1, 512], F32)
                        nc.vector.tensor_copy(sums_sb[:], x_t[0:1, :])
                        xs_sb = xssb.tile([Dh + 1, 512], BF16)
                        nc.vector.tensor_copy(xs_sb[:], x_ps)
                        # 4 PE transposes: sums [1,512] -> [128,4] PSUM
                        r_t = ops.tile([P, 512], F32, tag="ps512")
                        r_ps = r_t[:, 0:4]
                        for m in range(4):
                            nc.tensor.transpose(
                                r_ps[:, m : m + 1],
                                sums_sb[0:1, m * P : (m + 1) * P],
                                ident1[:])
                        rT = rsb.tile([P, 4], F32)
                        nc.vector.reciprocal(rT[:], r_ps)
                        out_sb = outpool.tile([P, 4, 512], F32)
                        for m in range(4):
                            out_ps = ops.tile([P, 512], F32, tag="ps512")
                            nc.tensor.matmul(out_ps[:],
                                             xs_sb[:, m * P : (m + 1) * P],
                                             wproj_sb[:],
                                             start=True, stop=True)
                            if m < n_act_ev:
                                nc.scalar.mul(out_sb[:, m, :], out_ps[:],
                                              rT[:, m : m + 1])
                            else:
                                nc.vector.tensor_scalar_mul(
                                    out_sb[:, m, :], out_ps[:],
                                    rT[:, m : m + 1])
                        row0 = b * S + qb * 512
                        nc.gpsimd.dma_start(
                            part[row0 : row0 + 512, :].rearrange(
                                "(m p) d -> p m d", p=P),
                            out_sb[:])

                    for qb in range(NQB):
                        if eb == 1:
                            aT_t = apool.tile([P, NKT, 512], BF16)
                            nc.sync.dma_start(aT_t[:], aT[0, qb])
                        for b in range(B):
                            if eb != 1:
                                aT_t = apool.tile([P, NKT, 512], BF16)
                                nc.sync.dma_start(aT_t[:], aT[b, qb])
                            emit_block(qb, b, aT_t)

    nc.compile()
    return nc


_CACHE = {}


def _get_program(eb: int):
    key = ("prog", eb)
    if key not in _CACHE:
        _CACHE[key] = build_program(eb)
    return _CACHE[key]


def prepare_inputs(hidden_states, attention_mask, alibi_bias, W_qkv, b_qkv,
                   W_proj, b_proj):
    """Host-side prep: transposes, scale folding, bf16 casts.
    Returns (in_maps, eb)."""
    hidden_states = np.asarray(hidden_states, dtype=np.float32)
    attention_mask = np.asarray(attention_mask)
    alibi_bias = np.asarray(alibi_bias, dtype=np.float32)
    W_qkv = np.asarray(W_qkv, dtype=np.float32)
    b_qkv = np.asarray(b_qkv, dtype=np.float32)
    W_proj = np.asarray(W_proj, dtype=np.float32)
    b_proj = np.asarray(b_proj, dtype=np.float32)

    # per-side scale: total scale SCALE, halved once more to undo the
    # duplicated-K (x2) trick in the score matmul
    s_side = np.float32(np.sqrt(SCALE / 2.0))

    hiddenT = np.ascontiguousarray(
        hidden_states.reshape(BS, D).T).astype(NP_BF16)

    mask_trivial = bool(attention_mask.all())
    eb = 1 if mask_trivial else B

    identity = np.eye(P, dtype=NP_BF16)

    def a_layout(m):
        # [S(k), S(q)] -> [NQB, 128, NKT, 512] contiguous per qb slice
        return np.ascontiguousarray(
            m.reshape(NKT, P, NQB, 512).transpose(2, 1, 0, 3))

    a_all = []
    for h in range(H):
        aTh = alibi_bias[0, h].T.astype(NP_BF16)  # [S(k), S(q)]
        if mask_trivial:
            a_all.append(a_layout(aTh)[None])
        else:
            a_all.append(np.stack(
                [a_layout(np.where(attention_mask[bi][:, None], aTh,
                                   NP_BF16(-30.0)))
                 for bi in range(B)]))
    in_maps = []
    for h in range(H):
        # reference reshapes qkv to (B, S, H, 3*Dh) then splits: head h's
        # q/k/v live in columns [h*3*Dh, h*3*Dh + 3*Dh)
        qs = slice(h * 3 * Dh, h * 3 * Dh + Dh)
        ks = slice(h * 3 * Dh + Dh, h * 3 * Dh + 2 * Dh)
        vs = slice(h * 3 * Dh + 2 * Dh, h * 3 * Dh + 3 * Dh)
        wqk_h = np.concatenate([W_qkv[:, qs], W_qkv[:, ks]], axis=1) * s_side
        bqk_h = np.concatenate([b_qkv[qs], b_qkv[ks]]) * s_side
        wv_h = W_qkv[:, vs]
        bv_h = b_qkv[vs]
        wproj_aug = np.concatenate(
            [(b_proj if h == 0 else np.zeros_like(b_proj))[None, :],
             W_proj[h * Dh : (h + 1) * Dh, :]], axis=0)
        in_maps.append({
            "hiddenT": hiddenT,
            "aT": a_all[h],
            "wqk": np.ascontiguousarray(
                wqk_h.reshape(4, P, P).astype(NP_BF16)),
            "bqk": np.ascontiguousarray(bqk_h[:, None]),
            "wv": np.ascontiguousarray(wv_h.reshape(4, P, Dh).astype(NP_BF16)),
            "bv": np.ascontiguousarray(bv_h[:, None]),
            "wproj": wproj_aug.astype(NP_BF16),
            "ident": identity,
        })
    return in_maps, eb


def kernel(**inputs):
    in_maps, eb = prepare_inputs(**inputs)
    nc = _get_program(eb)
    res = run_bass_kernel_spmd(nc, in_maps, list(range(H)))
    out = res.results[0]["part"].astype(np.float32)
    for h in range(1, H):
        out = out + res.results[h]["part"]
    return out.reshape(B, S, D)
